# revision 70
# baseline (speedup 1.0000x reference)
"""GPT-2-style causal attention block on 8 TRN2 NeuronCores (Bass/Tile).

Sharding (Megatron-style, per the hint): core c handles batch b = c // 4 and
head-group g = c % 4 (4 of the 16 heads).  Each core computes, fully locally:
  QKV projection (its 4 heads' columns), causal softmax attention for its
  4 heads, and the row-sharded output projection partial [S, D].
The host gathers by summing the 4 partials per batch and adding c_proj_b.

Per-core kernel layout choices:
  - x^T [D, S] is staged on host so Q^T/K^T come out of matmuls directly with
    head_dim on partitions (what the scores matmul wants) and V comes out in
    [seq, head_dim] (what the AV matmul wants).
  - scores are computed transposed, sT[j, i] (j = key index on partitions), so
    the exp'd tile is directly usable as the AV matmul's moving operand.
  - softmax uses exp without max subtraction (scores are O(1) here) and the
    denominator is computed by an extra ones-stationary matmul col-packed with
    the AV matmul, placed so numerator and denominator share partitions.
  - x/Wqkv/scores/probs/V run in bf16 (tile_position packing is illegal for
    4-byte dtypes); the output projection runs in float32r (TF32-rate).
  - attention is software-pipelined per (512-wide i-quarter, head pair):
    all score matmuls + one strided 2-head exp per key tile first, then the
    AV/rowsum accumulation chain, so PE never in-order-blocks behind ACT.
"""

from contextlib import ExitStack

import ml_dtypes
import numpy as np

B, S, D = 2, 2048, 1024
NH, HD = 16, 64
NCORES = 8
GROUPS = 4           # tensor-parallel head groups per batch
HPC = NH // GROUPS   # heads per core
SCALE = 1.0 / 8.0    # 1/sqrt(HD)

_CACHE = {}


def _body(ctx, tc, mybir, xt, wqk, wv, wp, qkb, vb, tri, onesb, onesr, out):
    nc = tc.nc
    f32 = mybir.dt.float32
    f32r = mybir.dt.float32r
    bf16 = mybir.dt.bfloat16
    EXP = mybir.ActivationFunctionType.Exp

    pin = ctx.enter_context(tc.tile_pool(name="pin", bufs=1))
    pwork = ctx.enter_context(tc.tile_pool(name="pwork", bufs=1))
    ppt = ctx.enter_context(tc.tile_pool(name="ppt", bufs=10))
    prec = ctx.enter_context(tc.tile_pool(name="prec", bufs=4))
    pstage = ctx.enter_context(tc.tile_pool(name="pstage", bufs=8))
    ps_mm = ctx.enter_context(tc.tile_pool(name="ps_mm", bufs=2, space="PSUM"))
    ps_s = ctx.enter_context(tc.tile_pool(name="ps_s", bufs=2, space="PSUM"))
    ps_av = ctx.enter_context(tc.tile_pool(name="ps_av", bufs=1, space="PSUM"))

    # ---------------- input staging ----------------
    # small tensors first (qkb gates the first bias add, tri the first
    # diagonal mask); wqk/xt interleaved per k-tile feed phase A as they
    # land; wv next (V fillers); wp last (not needed until the projection).
    qkb_sb = pin.tile([128, 4], f32, name="qkb_sb")
    tri_sb = pin.tile([128, 128], bf16, name="tri_sb")
    onesr_sb = pin.tile([128, 64], bf16, name="onesr_sb")
    vb_sb = pin.tile([128, 256], f32, name="vb_sb")
    xt_sb = pin.tile([128, 8 * 2048], bf16, name="xt_sb")
    wqk_sb = pin.tile([128, 4096], bf16, name="wqk_sb")
    for k in range(8):
        nc.sync.dma_start(wqk_sb[:, k * 512:(k + 1) * 512], wqk[:, k * 512:(k + 1) * 512])
        nc.sync.dma_start(xt_sb[:, k * 2048:(k + 1) * 2048], xt[k * 128:(k + 1) * 128, :])
        if k == 1:
            # small tensors ride between xt tiles: phase A has slack here
            nc.sync.dma_start(qkb_sb[:], qkb[:])
            nc.sync.dma_start(tri_sb[:], tri[:])
            nc.sync.dma_start(onesr_sb[:], onesr[:])
            nc.sync.dma_start(vb_sb[:], vb[:])
    wv_sb = pin.tile([128, 2048], bf16, name="wv_sb")
    wp_sb = pin.tile([128, 2048], f32r, name="wp_sb")

    # Q^T / K^T: head-pair p at cols [p*2048, (p+1)*2048); head hh of the pair
    # on partitions [hh*64, hh*64+64).
    qt_sb = pwork.tile([128, 2 * 2048], bf16, name="qt_sb")
    kt_sb = pwork.tile([128, 2 * 2048], bf16, name="kt_sb")
    # V: per j-tile 512 cols [V0 X V1 pad | V2 X V3 pad] where X is the
    # 64-col "denominator block" [1, 0x63].  Each AV stationary operand is a
    # plain contiguous 128-col window: [V_even|X] or [X|V_odd].  All 128
    # output rows are written from base 0: one half is the head's AV and the
    # X half also emits the softmax denominator (row 64 for even heads,
    # row 0 for odd ones) at zero extra PE time.
    v_sb = pwork.tile([128, 16 * 512], bf16, name="v_sb")
    # a^T: k2 (head pair) at cols [k2*2048, ...), head hh on partitions hh*64..
    at_sb = pwork.tile([128, 2 * 2048], f32r, name="at_sb")

    # onesb carries the tiled X pattern host-side
    for off in (64, 320):
        nc.sync.dma_start(
            v_sb.rearrange("p (j c) -> p j c", c=512)[:, :, off:off + 64],
            onesb.rearrange("p (j c) -> p j c", c=64))
    nc.sync.dma_start(wv_sb[:], wv[:])
    nc.sync.dma_start(wp_sb[:], wp[:])

    # Dummy exp so the ACT table set loads during the input-DMA window instead
    # of delaying the first real softmax exp.
    warm = pin.tile([128, 4], f32, name="warm")
    nc.scalar.activation(warm[:], qkb_sb[:], EXP, scale=0.0)



    # ---------------- QKV pair0: k-outer phase A ----------------
    # One matmul group per (Q/K, sc) with k outermost, so every arriving
    # x^T k-tile immediately feeds 8 x 512 cols of PE work: PE tracks the
    # input DMA stream instead of stalling on it.  The 8 accumulatorsborrow
    # all 8 PSUM banks (score slots hold two 512-wide groups side by side).
    sA = ps_s.tile([128, 1024], f32, tag="s", name="sA")
    sB = ps_s.tile([128, 1024], f32, tag="s", name="sB")
    kA = ps_av.tile([128, 512], f32, tag="ava", name="kA")
    kB = ps_av.tile([128, 512], f32, tag="avb", name="kB")
    kC = ps_mm.tile([128, 512], f32, tag="acc", name="kC")
    kD = ps_mm.tile([128, 512], f32, tag="acc", name="kD")
    qacc = [sA[:, 0:512], sA[:, 512:1024], sB[:, 0:512], sB[:, 512:1024]]
    kacc = [kA[:], kB[:], kC[:], kD[:]]
    for k in range(8):
        kw = dict(start=(k == 0), stop=(k == 7), skip_group_check=True)
        for sc in range(4):
            rhs = xt_sb[:, k * 2048 + sc * 512: k * 2048 + (sc + 1) * 512]
            nc.tensor.matmul(qacc[sc], lhsT=wqk_sb[:, k * 512: k * 512 + 128],
                             rhs=rhs, **kw)
            nc.tensor.matmul(kacc[sc], lhsT=wqk_sb[:, k * 512 + 256: k * 512 + 384],
                             rhs=rhs, **kw)
            if k == 7:
                # bias-add each sc as soon as its accumulation stops, so
                # att(0,0)'s first scores aren't gated on all four adds
                nc.vector.tensor_scalar_add(
                    qt_sb[:, sc * 512:(sc + 1) * 512], qacc[sc], qkb_sb[:, 0:1])
                nc.vector.tensor_scalar_add(
                    kt_sb[:, sc * 512:(sc + 1) * 512], kacc[sc], qkb_sb[:, 2:3])

    # QKV pair1, split into ~850ns filler chunks (2 k-tiles each) driven
    # inside the ACT-paced attention loops
    def qk1_fillers():
        out_f = []
        for sc in range(4):
            tiles = {}

            def chunk(kk, sc=sc, tiles=tiles):
                if kk == 0:
                    tiles[1] = ps_mm.tile([128, 512], f32, tag="acc", name=f"q1_{sc}")
                    tiles[3] = ps_mm.tile([128, 512], f32, tag="acc", name=f"k1_{sc}")
                for k in (2 * kk, 2 * kk + 1):
                    for C in (1, 3):
                        nc.tensor.matmul(
                            tiles[C][:],
                            lhsT=wqk_sb[:, k * 512 + C * 128: k * 512 + C * 128 + 128],
                            rhs=xt_sb[:, k * 2048 + sc * 512: k * 2048 + (sc + 1) * 512],
                            start=(k == 0), stop=(k == 7))
                if kk == 3:
                    nc.vector.tensor_scalar_add(
                        qt_sb[:, 2048 + sc * 512: 2048 + (sc + 1) * 512],
                        tiles[1][:], qkb_sb[:, 1:2])
                    nc.vector.tensor_scalar_add(
                        kt_sb[:, 2048 + sc * 512: 2048 + (sc + 1) * 512],
                        tiles[3][:], qkb_sb[:, 3:4])

            for kk in range(4):
                out_f.append(lambda kk=kk, chunk=chunk: chunk(kk))
        return out_f

    def v_jtile(j):
        ps = ps_mm.tile([128, 256], f32, tag="acc", name="ps_v")
        for k in range(8):
            nc.tensor.matmul(
                ps[:],
                lhsT=xt_sb[:, k * 2048 + j * 128: k * 2048 + (j + 1) * 128],
                rhs=wv_sb[:, k * 256:(k + 1) * 256],
                start=(k == 0), stop=(k == 7))
        nc.vector.tensor_add(
            v_sb[:, j * 512:(j + 1) * 512].rearrange(
                "p (a c) -> p a c", a=4)[:, :, 0:64],
            ps.rearrange("p (a c) -> p a c", a=4),
            vb_sb.rearrange("p (a c) -> p a c", a=4))

    # ---------------- attention ----------------
    # Processed per (i-quarter Q of 512, head-pair p).  Scores for both heads
    # of the pair share one [128, 1024] PSUM tile (head hh at cols hh*512), so
    # a single strided exp covers both.  AV + softmax-denominator matmuls are
    # col-packed into two PSUM banks:
    #   bank A: rows 0:64 = a~_h0 (V-MM),    rows 64:128 = rowsum_h1 (ones-MM)
    #   bank B: rows 0:64 = rowsum_h0,       rows 64:128 = a~_h1
    # so each head's numerator and denominator land on the same partitions.
    COPY = mybir.ActivationFunctionType.Copy

    def proj_half(st, ec, on_act=False):
        ps = ps_mm.tile([128, 512], f32, tag="acc", name="ps_o")
        for k2 in range(2):
            nc.tensor.matmul(
                ps[:],
                lhsT=at_sb[:, k2 * 2048 + st * 128: k2 * 2048 + (st + 1) * 128],
                rhs=wp_sb[:, k2 * 1024 + ec * 512: k2 * 1024 + (ec + 1) * 512],
                start=(k2 == 0), stop=(k2 == 1))
        stage = pstage.tile([128, 512], bf16, tag="stage", name="stage")
        if on_act:
            # tail path: ACT is idle after the last exp; keep DVE free for
            # the final normalize muls
            nc.scalar.activation(stage[:], ps[:], COPY)
        else:
            nc.vector.tensor_copy(stage[:], ps[:])
        nc.sync.dma_start(
            out[st * 128:(st + 1) * 128, ec * 512:(ec + 1) * 512], stage[:])

    DELAY = 3  # software-pipeline distance between scores/exp and AV use

    def att_qp(Q, p, norms, flush=None, tail=False):
        """Generator emitting scores/exp/AV for (Q, p), yielding once per J
        iteration so the driver can weave ~850ns PE filler chunks (V tiles,
        pair-1 QKV, projection halves) into the ACT-paced stretches.  Appends
        a closure that emits the softmax normalization (reciprocal + PE
        partition-broadcast + mul) to `norms`; callers flush it inside the
        NEXT att call (J==1) so the bcast matmuls never head-block score
        matmuls and the DVE reciprocal latency hides under them."""
        qlo = Q * 512
        Jmax = 4 * Q + 3
        nJ = 4 * Q + 4
        ava = ps_av.tile([128, 512], f32, tag="ava", name="ava")
        avb = ps_av.tile([128, 512], f32, tag="avb", name="avb")
        rec = prec.tile([128, 512], bf16, tag="rec", name="rec")
        h0 = p * 2
        pts = []
        for J in range(nJ + DELAY):
            if J < nJ:
                jlo = J * 128
                istart = max(jlo, qlo)
                w = qlo + 512 - istart
                pss = ps_s.tile([128, 1024], f32, tag="s", name="pss")
                for hh in range(2):
                    nc.tensor.matmul(
                        pss[:, hh * 512: hh * 512 + w],
                        lhsT=kt_sb[hh * 64:(hh + 1) * 64, p * 2048 + jlo: p * 2048 + jlo + 128],
                        rhs=qt_sb[hh * 64:(hh + 1) * 64, p * 2048 + istart: p * 2048 + istart + w],
                        start=True, stop=True)
                pt = ppt.tile([128, 1024], bf16, tag="pt", name="pt")
                nc.scalar.activation(
                    pt.rearrange("x (h c) -> x h c", c=512)[:, :, 0:w],
                    pss.rearrange("x (h c) -> x h c", c=512)[:, :, 0:w],
                    EXP, scale=SCALE)
                if jlo >= qlo:
                    # diagonal j-tile: zero the j > i triangle
                    nc.gpsimd.tensor_mul(pt[:, 0:128], pt[:, 0:128], tri_sb[:])
                    nc.gpsimd.tensor_mul(pt[:, 512:640], pt[:, 512:640], tri_sb[:])
                pts.append((pt, istart - qlo, w))
            if J == 0 and flush is not None:
                flush[0]()
            if J == 2 and flush is not None:
                flush[1]()
            Ja = J - DELAY
            if Ja < 0:
                yield
                continue
            pt, co, w = pts[Ja]
            r0 = pt[:, 0:w]
            r1 = pt[:, 512:512 + w]
            kw = dict(start=(Ja == 0), stop=(Ja == Jmax), skip_group_check=True)
            # [V_even|X]: rows 0:64 AV_h0, row 64 rowsum_h0;
            # [X|V_odd]:  row 0 rowsum_h1, rows 64:128 AV_h1.
            vb0 = Ja * 512 + p * 256
            nc.tensor.matmul(ava[:, co:512], lhsT=v_sb[:, vb0:vb0 + 128],
                             rhs=r0, **kw)
            nc.tensor.matmul(avb[:, co:512], lhsT=v_sb[:, vb0 + 64:vb0 + 192],
                             rhs=r1, **kw)
            if tail and Ja >= 4 * Q:
                # last quarter: each 128-col block of ava/avb is final once
                # its diagonal j-tile lands, so normalize + project that
                # block now, pipelined under the remaining AV/exp work
                d = Ja - 4 * Q
                lo, hi = 128 * d, 128 * (d + 1)
                if d == 0:
                    rc_t = ps_s.tile([128, 512], f32, tag="s", name="rc_t")
                    rcs_t = prec.tile([128, 512], f32, tag="rcs", name="rcs_t")
                with nc.allow_low_precision(reason="bf16 recip of softmax denom"):
                    nc.vector.reciprocal(rec[64:65, lo:hi], ava[64:65, lo:hi])
                    nc.vector.reciprocal(rec[0:1, lo:hi], avb[0:1, lo:hi])
                mmkw = dict(start=True, stop=True, skip_group_check=True)
                nc.tensor.matmul(rc_t[0:64, lo:hi], lhsT=onesr_sb[64:65, :],
                                 rhs=rec[64:65, lo:hi], **mmkw)
                nc.tensor.matmul(rc_t[64:128, lo:hi], lhsT=onesr_sb[0:1, :],
                                 rhs=rec[0:1, lo:hi], **mmkw)
                nc.vector.tensor_copy(rcs_t[:, lo:hi], rc_t[:, lo:hi])
                base = p * 2048 + qlo
                nc.vector.tensor_mul(at_sb[0:64, base + lo: base + hi],
                                     ava[0:64, lo:hi], rcs_t[0:64, lo:hi])
                nc.vector.tensor_mul(at_sb[64:128, base + lo: base + hi],
                                     avb[64:128, lo:hi], rcs_t[64:128, lo:hi])
                proj_half(4 * Q + d, 0, on_act=True)
                proj_half(4 * Q + d, 1, on_act=True)
            yield

        def norm_recip():
            # reciprocal of the single denominator rows, emitted one flush
            # point earlier than the broadcast so its DVE latency is hidden
            with nc.allow_low_precision(reason="bf16 reciprocal of softmax denom"):
                nc.vector.reciprocal(rec[64:65, :], ava[64:65, :])
                nc.vector.reciprocal(rec[0:1, :], avb[0:1, :])

        def norm(tail_cb=None, rc_pool=None, rc_tag="acc"):
            # PE broadcasts 1/denom across the 64 hd partitions (bf16 lhsT)
            rc = (rc_pool or ps_mm).tile([128, 512], f32, tag=rc_tag, name="rc")
            nc.tensor.matmul(rc[0:64, :], lhsT=onesr_sb[64:65, :],
                             rhs=rec[64:65, :], start=True, stop=True)
            nc.tensor.matmul(rc[64:128, :], lhsT=onesr_sb[0:1, :],
                             rhs=rec[0:1, :], start=True, stop=True)
            # DVE can't read two PSUM operands in one op: stage rc in SBUF
            rcs = prec.tile([128, 512], f32, tag="rcs", name="rcs")
            nc.vector.tensor_copy(rcs[:], rc[:])
            halves = (0, 1) if tail_cb is not None else (None,)
            for half in halves:
                cs = slice(0, 512) if half is None else slice(half * 256, half * 256 + 256)
                nc.vector.tensor_mul(
                    at_sb[0:64, p * 2048 + qlo + cs.start: p * 2048 + qlo + cs.stop],
                    ava[0:64, cs], rcs[0:64, cs])
                nc.vector.tensor_mul(
                    at_sb[64:128, p * 2048 + qlo + cs.start: p * 2048 + qlo + cs.stop],
                    avb[64:128, cs], rcs[64:128, cs])
                if tail_cb is not None:
                    tail_cb(half)

        norms.append(None if tail else (norm_recip, norm))

    # ---------------- driver: attention with woven fillers ----------------
    fillers = []

    def run_att(Q, p, flush=None, max_pops=None, tail=False):
        holder = []
        pops = 0
        for _ in att_qp(Q, p, holder, flush, tail=tail):
            if fillers and (max_pops is None or pops < max_pops):
                fillers.pop(0)()
                pops += 1
        return holder[0]

    def queue_proj_after(nrm, sts):
        # run the pending normalize, then queue the projection halves that
        # depend on it (they must not be emitted before at~ is written)
        def f():
            nrm[1]()
            fillers.extend(
                lambda st=st, ec=ec: proj_half(st, ec)
                for st in sts for ec in range(2))
        return (nrm[0], f)

    qk1 = qk1_fillers()
    # filler distribution pushes work toward the late, ACT-paced quarters:
    # V j before the first AV that consumes it, qk pair-1 sc blocks just
    # before the att(*, 1) quarter that reads them, projections after their
    # quarter's normalize
    fillers.extend(lambda j=j: v_jtile(j) for j in range(4))
    n00 = run_att(0, 0)
    fillers.extend(lambda j=j: v_jtile(j) for j in range(4, 8))
    fillers.extend(qk1[0:4])
    n10 = run_att(1, 0, flush=n00)
    fillers.extend(qk1[4:8])
    n01 = run_att(0, 1, flush=n10)
    n11 = run_att(1, 1, flush=queue_proj_after(n01, range(0, 4)))
    # (2,1)/(3,1) read qt pair-1 sc2/sc3 at J=0, so those qk1 blocks weave
    # into the preceding pair-0 quarter; max_pops holds back the projection
    # fillers so they land in the otherwise-empty att(*, 1) stretches
    fillers.extend(qk1[8:12])
    fillers.extend(lambda j=j: v_jtile(j) for j in range(8, 12))
    n20 = run_att(2, 0, flush=queue_proj_after(n11, range(4, 8)), max_pops=2)
    n21 = run_att(2, 1, flush=n20)
    fillers.extend(qk1[12:16])
    fillers.extend(lambda j=j: v_jtile(j) for j in range(12, 16))
    n30 = run_att(3, 0, flush=queue_proj_after(n21, range(8, 12)), max_pops=2)
    run_att(3, 1, flush=n30, tail=True)
    while fillers:
        fillers.pop(0)()

def _build_nc(repeat=1):
    key = ("nc", repeat)
    if key in _CACHE:
        return _CACHE[key]
    import concourse.bacc as bacc
    import concourse.mybir as mybir
    import concourse.tile as tile

    f32 = mybir.dt.float32
    f32r = mybir.dt.float32r
    bf16d = mybir.dt.bfloat16
    nc = bacc.Bacc("TRN2", target_bir_lowering=False, debug=False)
    xt = nc.dram_tensor("xt", [D, S], bf16d, kind="ExternalInput").ap()
    wqk = nc.dram_tensor("wqk", [128, 4096], bf16d, kind="ExternalInput").ap()
    wv = nc.dram_tensor("wv", [128, 2048], bf16d, kind="ExternalInput").ap()
    wp = nc.dram_tensor("wp", [128, 2048], f32r, kind="ExternalInput").ap()
    qkb = nc.dram_tensor("qkb", [128, 4], f32, kind="ExternalInput").ap()
    vb = nc.dram_tensor("vb", [128, 256], f32, kind="ExternalInput").ap()
    tri = nc.dram_tensor("tri", [128, 128], mybir.dt.bfloat16, kind="ExternalInput").ap()
    onesb = nc.dram_tensor("onesb", [128, 1024], mybir.dt.bfloat16, kind="ExternalInput").ap()
    onesr = nc.dram_tensor("onesr", [128, 64], mybir.dt.bfloat16, kind="ExternalInput").ap()
    out = nc.dram_tensor("out", [S, D], bf16d, kind="ExternalOutput").ap()

    with tile.TileContext(nc) as tc:
        for _ in range(repeat):
            with ExitStack() as ctx:
                _body(ctx, tc, mybir, xt, wqk, wv, wp, qkb, vb, tri, onesb, onesr, out)
    nc.compile()
    _CACHE[key] = nc
    return nc


def _make_in_maps(hidden_states, c_attn_w, c_attn_b, c_proj_w):
    hs = np.asarray(hidden_states, dtype=np.float32)
    waw = np.asarray(c_attn_w, dtype=np.float32)
    wab = np.asarray(c_attn_b, dtype=np.float32)
    wpw = np.asarray(c_proj_w, dtype=np.float32)

    tri = np.triu(np.ones((128, 128), dtype=ml_dtypes.bfloat16))
    # tiled 64-col denominator-block pattern [1, 0x31, 1, 0x31]
    dpat = np.zeros(64, np.float32)
    dpat[0] = 1
    onesb_host = np.broadcast_to(
        np.tile(dpat, 16), (128, 1024)).astype(ml_dtypes.bfloat16)
    xts = [np.ascontiguousarray(hs[b].T).astype(ml_dtypes.bfloat16) for b in range(B)]
    in_maps = []
    for c in range(NCORES):
        b, g = divmod(c, GROUPS)
        cols = np.arange(g * HPC * HD, (g + 1) * HPC * HD)
        wqk_host = np.concatenate([waw[:, cols], waw[:, D + cols]], axis=1)
        in_maps.append({
            "xt": xts[b],
            "wqk": np.ascontiguousarray(
                wqk_host.reshape(8, 128, 512).transpose(1, 0, 2).reshape(128, 4096)).astype(ml_dtypes.bfloat16),
            "wv": np.ascontiguousarray(
                waw[:, 2 * D + cols].reshape(8, 128, 256).transpose(1, 0, 2).reshape(128, 2048)).astype(ml_dtypes.bfloat16),
            "wp": np.ascontiguousarray(
                wpw[cols, :].reshape(2, 128, 1024).transpose(1, 0, 2).reshape(128, 2048)),
            "qkb": np.ascontiguousarray(
                np.concatenate([wab[cols], wab[D + cols]]).reshape(4, 128).T),
            "vb": np.ascontiguousarray(
                np.broadcast_to(wab[2 * D + cols], (128, 256))),
            "tri": tri,
            "onesb": onesb_host,
            "onesr": np.ones((128, 64), ml_dtypes.bfloat16),
        })
    return in_maps


def kernel(hidden_states, c_attn_w, c_attn_b, c_proj_w, c_proj_b):
    from concourse import bass_utils

    nc = _build_nc()
    in_maps = _make_in_maps(hidden_states, c_attn_w, c_attn_b, c_proj_w)
    res = bass_utils.run_bass_kernel_spmd(nc, in_maps, core_ids=list(range(NCORES)))
    outs = [np.asarray(r["out"], dtype=np.float32) for r in res.results]
    wpb = np.asarray(c_proj_b, dtype=np.float32)
    full = np.stack(
        [sum(outs[b * GROUPS:(b + 1) * GROUPS]) + wpb for b in range(B)], axis=0)
    return full.astype(np.float32)



# revision 71
# speedup vs baseline: 1.0308x; 1.0308x over previous
"""GPT-2-style causal attention block on 8 TRN2 NeuronCores (Bass/Tile).

Sharding (Megatron-style, per the hint): core c handles batch b = c // 4 and
head-group g = c % 4 (4 of the 16 heads).  Each core computes, fully locally:
  QKV projection (its 4 heads' columns), causal softmax attention for its
  4 heads, and the row-sharded output projection partial [S, D].
The host gathers by summing the 4 partials per batch and adding c_proj_b.

Per-core kernel layout choices:
  - x^T [D, S] is staged on host so Q^T/K^T come out of matmuls directly with
    head_dim on partitions (what the scores matmul wants) and V comes out in
    [seq, head_dim] (what the AV matmul wants).
  - scores are computed transposed, sT[j, i] (j = key index on partitions), so
    the exp'd tile is directly usable as the AV matmul's moving operand.
  - softmax uses exp without max subtraction (scores are O(1) here) and the
    denominator is computed by an extra ones-stationary matmul col-packed with
    the AV matmul, placed so numerator and denominator share partitions.
  - x/Wqkv/scores/probs/V run in bf16 (tile_position packing is illegal for
    4-byte dtypes); the output projection runs in float32r (TF32-rate).
  - attention is software-pipelined per (512-wide i-quarter, head pair):
    all score matmuls + one strided 2-head exp per key tile first, then the
    AV/rowsum accumulation chain, so PE never in-order-blocks behind ACT.
"""

from contextlib import ExitStack

import ml_dtypes
import numpy as np

B, S, D = 2, 2048, 1024
NH, HD = 16, 64
NCORES = 8
GROUPS = 4           # tensor-parallel head groups per batch
HPC = NH // GROUPS   # heads per core
SCALE = 1.0 / 8.0    # 1/sqrt(HD)

_CACHE = {}


def _body(ctx, tc, mybir, xt, wqk, wv, wp, qkb, vb, tri, onesb, onesr, out):
    nc = tc.nc
    f32 = mybir.dt.float32
    f32r = mybir.dt.float32r
    bf16 = mybir.dt.bfloat16
    EXP = mybir.ActivationFunctionType.Exp

    pin = ctx.enter_context(tc.tile_pool(name="pin", bufs=1))
    pwork = ctx.enter_context(tc.tile_pool(name="pwork", bufs=1))
    ppt = ctx.enter_context(tc.tile_pool(name="ppt", bufs=10))
    prec = ctx.enter_context(tc.tile_pool(name="prec", bufs=4))
    pstage = ctx.enter_context(tc.tile_pool(name="pstage", bufs=8))
    ps_mm = ctx.enter_context(tc.tile_pool(name="ps_mm", bufs=2, space="PSUM"))
    ps_s = ctx.enter_context(tc.tile_pool(name="ps_s", bufs=2, space="PSUM"))
    ps_av = ctx.enter_context(tc.tile_pool(name="ps_av", bufs=1, space="PSUM"))

    # ---------------- input staging ----------------
    # small tensors first (qkb gates the first bias add, tri the first
    # diagonal mask); wqk/xt interleaved per k-tile feed phase A as they
    # land; wv next (V fillers); wp last (not needed until the projection).
    qkb_sb = pin.tile([128, 4], f32, name="qkb_sb")
    tri_sb = pin.tile([128, 128], bf16, name="tri_sb")
    onesr_sb = pin.tile([128, 64], bf16, name="onesr_sb")
    vb_sb = pin.tile([128, 256], f32, name="vb_sb")
    xt_sb = pin.tile([128, 8 * 2048], bf16, name="xt_sb")
    wqk_sb = pin.tile([128, 4096], bf16, name="wqk_sb")
    for k in range(8):
        nc.sync.dma_start(wqk_sb[:, k * 512:(k + 1) * 512], wqk[:, k * 512:(k + 1) * 512])
        nc.sync.dma_start(xt_sb[:, k * 2048:(k + 1) * 2048], xt[k * 128:(k + 1) * 128, :])
        if k == 1:
            # small tensors ride between xt tiles: phase A has slack here
            nc.sync.dma_start(qkb_sb[:], qkb[:])
            nc.sync.dma_start(tri_sb[:], tri[:])
            nc.sync.dma_start(onesr_sb[:], onesr[:])
            nc.sync.dma_start(vb_sb[:], vb[:])
    wv_sb = pin.tile([128, 2048], bf16, name="wv_sb")
    wp_sb = pin.tile([128, 2048], f32r, name="wp_sb")

    # Q^T / K^T: head-pair p at cols [p*2048, (p+1)*2048); head hh of the pair
    # on partitions [hh*64, hh*64+64).
    qt_sb = pwork.tile([128, 2 * 2048], bf16, name="qt_sb")
    kt_sb = pwork.tile([128, 2 * 2048], bf16, name="kt_sb")
    # V: per j-tile 512 cols [V0 X V1 pad | V2 X V3 pad] where X is the
    # 64-col "denominator block" [1, 0x63].  Each AV stationary operand is a
    # plain contiguous 128-col window: [V_even|X] or [X|V_odd].  All 128
    # output rows are written from base 0: one half is the head's AV and the
    # X half also emits the softmax denominator (row 64 for even heads,
    # row 0 for odd ones) at zero extra PE time.
    v_sb = pwork.tile([128, 16 * 512], bf16, name="v_sb")
    # a^T: k2 (head pair) at cols [k2*2048, ...), head hh on partitions hh*64..
    at_sb = pwork.tile([128, 2 * 2048], f32r, name="at_sb")

    # onesb carries the tiled X pattern host-side
    for off in (64, 320):
        nc.sync.dma_start(
            v_sb.rearrange("p (j c) -> p j c", c=512)[:, :, off:off + 64],
            onesb.rearrange("p (j c) -> p j c", c=64))
    nc.sync.dma_start(wv_sb[:], wv[:])
    nc.sync.dma_start(wp_sb[:], wp[:])

    # Dummy exp so the ACT table set loads during the input-DMA window instead
    # of delaying the first real softmax exp.
    warm = pin.tile([128, 4], f32, name="warm")
    nc.scalar.activation(warm[:], qkb_sb[:], EXP, scale=0.0)



    # ---------------- QKV pair0: k-outer phase A ----------------
    # One matmul group per (Q/K, sc) with k outermost, so every arriving
    # x^T k-tile immediately feeds 8 x 512 cols of PE work: PE tracks the
    # input DMA stream instead of stalling on it.  The 8 accumulatorsborrow
    # all 8 PSUM banks (score slots hold two 512-wide groups side by side).
    sA = ps_s.tile([128, 1024], f32, tag="s", name="sA")
    sB = ps_s.tile([128, 1024], f32, tag="s", name="sB")
    kA = ps_av.tile([128, 512], f32, tag="ava", name="kA")
    kB = ps_av.tile([128, 512], f32, tag="avb", name="kB")
    kC = ps_mm.tile([128, 512], f32, tag="acc", name="kC")
    kD = ps_mm.tile([128, 512], f32, tag="acc", name="kD")
    qacc = [sA[:, 0:512], sA[:, 512:1024], sB[:, 0:512], sB[:, 512:1024]]
    kacc = [kA[:], kB[:], kC[:], kD[:]]
    for k in range(8):
        kw = dict(start=(k == 0), stop=(k == 7), skip_group_check=True)
        for sc in range(4):
            rhs = xt_sb[:, k * 2048 + sc * 512: k * 2048 + (sc + 1) * 512]
            nc.tensor.matmul(qacc[sc], lhsT=wqk_sb[:, k * 512: k * 512 + 128],
                             rhs=rhs, **kw)
            nc.tensor.matmul(kacc[sc], lhsT=wqk_sb[:, k * 512 + 256: k * 512 + 384],
                             rhs=rhs, **kw)
            if k == 7:
                # bias-add each sc as soon as its accumulation stops, so
                # att(0,0)'s first scores aren't gated on all four adds
                nc.vector.tensor_scalar_add(
                    qt_sb[:, sc * 512:(sc + 1) * 512], qacc[sc], qkb_sb[:, 0:1])
                nc.vector.tensor_scalar_add(
                    kt_sb[:, sc * 512:(sc + 1) * 512], kacc[sc], qkb_sb[:, 2:3])

    # QKV pair1, split into ~850ns filler chunks (2 k-tiles each) driven
    # inside the ACT-paced attention loops
    def qk1_fillers():
        out_f = []
        for sc in range(4):
            tiles = {}

            def chunk(kk, sc=sc, tiles=tiles):
                if kk == 0:
                    tiles[1] = ps_mm.tile([128, 512], f32, tag="acc", name=f"q1_{sc}")
                    tiles[3] = ps_mm.tile([128, 512], f32, tag="acc", name=f"k1_{sc}")
                for k in (2 * kk, 2 * kk + 1):
                    for C in (1, 3):
                        nc.tensor.matmul(
                            tiles[C][:],
                            lhsT=wqk_sb[:, k * 512 + C * 128: k * 512 + C * 128 + 128],
                            rhs=xt_sb[:, k * 2048 + sc * 512: k * 2048 + (sc + 1) * 512],
                            start=(k == 0), stop=(k == 7))
                if kk == 3:
                    nc.vector.tensor_scalar_add(
                        qt_sb[:, 2048 + sc * 512: 2048 + (sc + 1) * 512],
                        tiles[1][:], qkb_sb[:, 1:2])
                    nc.vector.tensor_scalar_add(
                        kt_sb[:, 2048 + sc * 512: 2048 + (sc + 1) * 512],
                        tiles[3][:], qkb_sb[:, 3:4])

            for kk in range(4):
                out_f.append(lambda kk=kk, chunk=chunk: chunk(kk))
        return out_f

    def v_jtile(j):
        ps = ps_mm.tile([128, 256], f32, tag="acc", name="ps_v")
        for k in range(8):
            nc.tensor.matmul(
                ps[:],
                lhsT=xt_sb[:, k * 2048 + j * 128: k * 2048 + (j + 1) * 128],
                rhs=wv_sb[:, k * 256:(k + 1) * 256],
                start=(k == 0), stop=(k == 7))
        nc.vector.tensor_add(
            v_sb[:, j * 512:(j + 1) * 512].rearrange(
                "p (a c) -> p a c", a=4)[:, :, 0:64],
            ps.rearrange("p (a c) -> p a c", a=4),
            vb_sb.rearrange("p (a c) -> p a c", a=4))

    # ---------------- attention ----------------
    # Processed per (i-quarter Q of 512, head-pair p).  Scores for both heads
    # of the pair share one [128, 1024] PSUM tile (head hh at cols hh*512), so
    # a single strided exp covers both.  AV + softmax-denominator matmuls are
    # col-packed into two PSUM banks:
    #   bank A: rows 0:64 = a~_h0 (V-MM),    rows 64:128 = rowsum_h1 (ones-MM)
    #   bank B: rows 0:64 = rowsum_h0,       rows 64:128 = a~_h1
    # so each head's numerator and denominator land on the same partitions.
    COPY = mybir.ActivationFunctionType.Copy

    def proj_half(st, ec, on_act=False):
        ps = ps_mm.tile([128, 512], f32, tag="acc", name="ps_o")
        for k2 in range(2):
            nc.tensor.matmul(
                ps[:],
                lhsT=at_sb[:, k2 * 2048 + st * 128: k2 * 2048 + (st + 1) * 128],
                rhs=wp_sb[:, k2 * 1024 + ec * 512: k2 * 1024 + (ec + 1) * 512],
                start=(k2 == 0), stop=(k2 == 1))
        stage = pstage.tile([128, 512], bf16, tag="stage", name="stage")
        if on_act:
            # tail path: ACT is idle after the last exp; keep DVE free for
            # the final normalize muls
            nc.scalar.activation(stage[:], ps[:], COPY)
        else:
            nc.vector.tensor_copy(stage[:], ps[:])
        nc.sync.dma_start(
            out[st * 128:(st + 1) * 128, ec * 512:(ec + 1) * 512], stage[:])

    DELAY = 3  # software-pipeline distance between scores/exp and AV use

    def att_qp(Q, p, norms, flush=None, tail=False):
        """Generator emitting scores/exp/AV for (Q, p), yielding once per J
        iteration so the driver can weave ~850ns PE filler chunks (V tiles,
        pair-1 QKV, projection halves) into the ACT-paced stretches.  Appends
        a closure that emits the softmax normalization (reciprocal + PE
        partition-broadcast + mul) to `norms`; callers flush it inside the
        NEXT att call (J==1) so the bcast matmuls never head-block score
        matmuls and the DVE reciprocal latency hides under them."""
        qlo = Q * 512
        Jmax = 4 * Q + 3
        nJ = 4 * Q + 4
        ava = ps_av.tile([128, 512], f32, tag="ava", name="ava")
        avb = ps_av.tile([128, 512], f32, tag="avb", name="avb")
        rec = prec.tile([128, 512], bf16, tag="rec", name="rec")
        h0 = p * 2
        pts = []
        for J in range(nJ + DELAY):
            if J < nJ:
                jlo = J * 128
                istart = max(jlo, qlo)
                w = qlo + 512 - istart
                pss = ps_s.tile([128, 1024], f32, tag="s", name="pss")
                for hh in range(2):
                    nc.tensor.matmul(
                        pss[:, hh * 512: hh * 512 + w],
                        lhsT=kt_sb[hh * 64:(hh + 1) * 64, p * 2048 + jlo: p * 2048 + jlo + 128],
                        rhs=qt_sb[hh * 64:(hh + 1) * 64, p * 2048 + istart: p * 2048 + istart + w],
                        start=True, stop=True)
                pt = ppt.tile([128, 1024], bf16, tag="pt", name="pt")
                nc.scalar.activation(
                    pt.rearrange("x (h c) -> x h c", c=512)[:, :, 0:w],
                    pss.rearrange("x (h c) -> x h c", c=512)[:, :, 0:w],
                    EXP, scale=SCALE)
                if jlo >= qlo:
                    # diagonal j-tile: zero the j > i triangle
                    nc.gpsimd.tensor_mul(pt[:, 0:128], pt[:, 0:128], tri_sb[:])
                    nc.gpsimd.tensor_mul(pt[:, 512:640], pt[:, 512:640], tri_sb[:])
                pts.append((pt, istart - qlo, w))
            if J == 0 and flush is not None:
                flush[0]()
            if J == 2 and flush is not None:
                flush[1]()
            Ja = J - DELAY
            if Ja < 0:
                yield
                continue
            pt, co, w = pts[Ja]
            r0 = pt[:, 0:w]
            r1 = pt[:, 512:512 + w]
            kw = dict(start=(Ja == 0), stop=(Ja == Jmax), skip_group_check=True)
            # [V_even|X]: rows 0:64 AV_h0, row 64 rowsum_h0;
            # [X|V_odd]:  row 0 rowsum_h1, rows 64:128 AV_h1.
            vb0 = Ja * 512 + p * 256
            nc.tensor.matmul(ava[:, co:512], lhsT=v_sb[:, vb0:vb0 + 128],
                             rhs=r0, **kw)
            nc.tensor.matmul(avb[:, co:512], lhsT=v_sb[:, vb0 + 64:vb0 + 192],
                             rhs=r1, **kw)
            if tail and Ja >= 4 * Q:
                # last quarter: each 128-col block of ava/avb is final once
                # its diagonal j-tile lands, so normalize + project that
                # block now, pipelined under the remaining AV/exp work
                d = Ja - 4 * Q
                lo, hi = 128 * d, 128 * (d + 1)
                if d == 0:
                    rc_t = ps_s.tile([128, 512], f32, tag="s", name="rc_t")
                    rcs_t = prec.tile([128, 512], f32, tag="rcs", name="rcs_t")
                with nc.allow_low_precision(reason="bf16 recip of softmax denom"):
                    nc.vector.reciprocal(rec[64:65, lo:hi], ava[64:65, lo:hi])
                    nc.vector.reciprocal(rec[0:1, lo:hi], avb[0:1, lo:hi])
                mmkw = dict(start=True, stop=True, skip_group_check=True)
                nc.tensor.matmul(rc_t[0:64, lo:hi], lhsT=onesr_sb[64:65, :],
                                 rhs=rec[64:65, lo:hi], **mmkw)
                nc.tensor.matmul(rc_t[64:128, lo:hi], lhsT=onesr_sb[0:1, :],
                                 rhs=rec[0:1, lo:hi], **mmkw)
                nc.vector.tensor_copy(rcs_t[:, lo:hi], rc_t[:, lo:hi])
                base = p * 2048 + qlo
                nc.vector.tensor_mul(at_sb[0:64, base + lo: base + hi],
                                     ava[0:64, lo:hi], rcs_t[0:64, lo:hi])
                nc.vector.tensor_mul(at_sb[64:128, base + lo: base + hi],
                                     avb[64:128, lo:hi], rcs_t[64:128, lo:hi])
                proj_half(4 * Q + d, 0, on_act=True)
                proj_half(4 * Q + d, 1, on_act=True)
            yield

        def norm_recip():
            # reciprocal of the single denominator rows, emitted one flush
            # point earlier than the broadcast so its DVE latency is hidden
            with nc.allow_low_precision(reason="bf16 reciprocal of softmax denom"):
                nc.vector.reciprocal(rec[64:65, :], ava[64:65, :])
                nc.vector.reciprocal(rec[0:1, :], avb[0:1, :])

        def norm(tail_cb=None, rc_pool=None, rc_tag="acc"):
            # PE broadcasts 1/denom across the 64 hd partitions (bf16 lhsT)
            rc = (rc_pool or ps_mm).tile([128, 512], f32, tag=rc_tag, name="rc")
            nc.tensor.matmul(rc[0:64, :], lhsT=onesr_sb[64:65, :],
                             rhs=rec[64:65, :], start=True, stop=True)
            nc.tensor.matmul(rc[64:128, :], lhsT=onesr_sb[0:1, :],
                             rhs=rec[0:1, :], start=True, stop=True)
            # DVE can't read two PSUM operands in one op: stage rc in SBUF
            rcs = prec.tile([128, 512], f32, tag="rcs", name="rcs")
            nc.vector.tensor_copy(rcs[:], rc[:])
            halves = (0, 1) if tail_cb is not None else (None,)
            for half in halves:
                cs = slice(0, 512) if half is None else slice(half * 256, half * 256 + 256)
                nc.vector.tensor_mul(
                    at_sb[0:64, p * 2048 + qlo + cs.start: p * 2048 + qlo + cs.stop],
                    ava[0:64, cs], rcs[0:64, cs])
                nc.vector.tensor_mul(
                    at_sb[64:128, p * 2048 + qlo + cs.start: p * 2048 + qlo + cs.stop],
                    avb[64:128, cs], rcs[64:128, cs])
                if tail_cb is not None:
                    tail_cb(half)

        norms.append(None if tail else (norm_recip, norm))

    # ---------------- driver: attention with woven fillers ----------------
    fillers = []

    def run_att(Q, p, flush=None, max_pops=None, tail=False):
        holder = []
        pops = 0
        for _ in att_qp(Q, p, holder, flush, tail=tail):
            if fillers and (max_pops is None or pops < max_pops):
                fillers.pop(0)()
                pops += 1
        return holder[0]

    def queue_proj_after(nrm, sts):
        # run the pending normalize, then queue the projection halves that
        # depend on it (they must not be emitted before at~ is written)
        def f():
            nrm[1]()
            fillers.extend(
                lambda st=st, ec=ec: proj_half(st, ec)
                for st in sts for ec in range(2))
        return (nrm[0], f)

    qk1 = qk1_fillers()
    # filler distribution pushes work toward the late, ACT-paced quarters:
    # V j before the first AV that consumes it, qk pair-1 sc blocks just
    # before the att(*, 1) quarter that reads them, projections after their
    # quarter's normalize
    fillers.extend(lambda j=j: v_jtile(j) for j in range(4))
    n00 = run_att(0, 0)
    fillers.extend(lambda j=j: v_jtile(j) for j in range(4, 8))
    fillers.extend(qk1[0:4])
    n10 = run_att(1, 0, flush=n00)
    fillers.extend(qk1[4:8])
    n01 = run_att(0, 1, flush=n10)
    n11 = run_att(1, 1, flush=queue_proj_after(n01, range(0, 4)))
    # (2,1)/(3,1) read qt pair-1 sc2/sc3 at J=0, so those qk1 blocks weave
    # into the preceding pair-0 quarter; max_pops holds back the projection
    # fillers so they land in the otherwise-empty att(*, 1) stretches
    fillers.extend(qk1[8:12])
    fillers.extend(lambda j=j: v_jtile(j) for j in range(8, 12))
    n20 = run_att(2, 0, flush=queue_proj_after(n11, range(4, 8)), max_pops=5)
    n21 = run_att(2, 1, flush=n20)
    fillers.extend(qk1[12:16])
    fillers.extend(lambda j=j: v_jtile(j) for j in range(12, 16))
    n30 = run_att(3, 0, flush=queue_proj_after(n21, range(8, 12)), max_pops=5)
    run_att(3, 1, flush=n30, tail=True)
    while fillers:
        fillers.pop(0)()

def _build_nc(repeat=1):
    key = ("nc", repeat)
    if key in _CACHE:
        return _CACHE[key]
    import concourse.bacc as bacc
    import concourse.mybir as mybir
    import concourse.tile as tile

    f32 = mybir.dt.float32
    f32r = mybir.dt.float32r
    bf16d = mybir.dt.bfloat16
    nc = bacc.Bacc("TRN2", target_bir_lowering=False, debug=False)
    xt = nc.dram_tensor("xt", [D, S], bf16d, kind="ExternalInput").ap()
    wqk = nc.dram_tensor("wqk", [128, 4096], bf16d, kind="ExternalInput").ap()
    wv = nc.dram_tensor("wv", [128, 2048], bf16d, kind="ExternalInput").ap()
    wp = nc.dram_tensor("wp", [128, 2048], f32r, kind="ExternalInput").ap()
    qkb = nc.dram_tensor("qkb", [128, 4], f32, kind="ExternalInput").ap()
    vb = nc.dram_tensor("vb", [128, 256], f32, kind="ExternalInput").ap()
    tri = nc.dram_tensor("tri", [128, 128], mybir.dt.bfloat16, kind="ExternalInput").ap()
    onesb = nc.dram_tensor("onesb", [128, 1024], mybir.dt.bfloat16, kind="ExternalInput").ap()
    onesr = nc.dram_tensor("onesr", [128, 64], mybir.dt.bfloat16, kind="ExternalInput").ap()
    out = nc.dram_tensor("out", [S, D], bf16d, kind="ExternalOutput").ap()

    with tile.TileContext(nc) as tc:
        for _ in range(repeat):
            with ExitStack() as ctx:
                _body(ctx, tc, mybir, xt, wqk, wv, wp, qkb, vb, tri, onesb, onesr, out)
    nc.compile()
    _CACHE[key] = nc
    return nc


def _make_in_maps(hidden_states, c_attn_w, c_attn_b, c_proj_w):
    hs = np.asarray(hidden_states, dtype=np.float32)
    waw = np.asarray(c_attn_w, dtype=np.float32)
    wab = np.asarray(c_attn_b, dtype=np.float32)
    wpw = np.asarray(c_proj_w, dtype=np.float32)

    tri = np.triu(np.ones((128, 128), dtype=ml_dtypes.bfloat16))
    # tiled 64-col denominator-block pattern [1, 0x31, 1, 0x31]
    dpat = np.zeros(64, np.float32)
    dpat[0] = 1
    onesb_host = np.broadcast_to(
        np.tile(dpat, 16), (128, 1024)).astype(ml_dtypes.bfloat16)
    xts = [np.ascontiguousarray(hs[b].T).astype(ml_dtypes.bfloat16) for b in range(B)]
    in_maps = []
    for c in range(NCORES):
        b, g = divmod(c, GROUPS)
        cols = np.arange(g * HPC * HD, (g + 1) * HPC * HD)
        wqk_host = np.concatenate([waw[:, cols], waw[:, D + cols]], axis=1)
        in_maps.append({
            "xt": xts[b],
            "wqk": np.ascontiguousarray(
                wqk_host.reshape(8, 128, 512).transpose(1, 0, 2).reshape(128, 4096)).astype(ml_dtypes.bfloat16),
            "wv": np.ascontiguousarray(
                waw[:, 2 * D + cols].reshape(8, 128, 256).transpose(1, 0, 2).reshape(128, 2048)).astype(ml_dtypes.bfloat16),
            "wp": np.ascontiguousarray(
                wpw[cols, :].reshape(2, 128, 1024).transpose(1, 0, 2).reshape(128, 2048)),
            "qkb": np.ascontiguousarray(
                np.concatenate([wab[cols], wab[D + cols]]).reshape(4, 128).T),
            "vb": np.ascontiguousarray(
                np.broadcast_to(wab[2 * D + cols], (128, 256))),
            "tri": tri,
            "onesb": onesb_host,
            "onesr": np.ones((128, 64), ml_dtypes.bfloat16),
        })
    return in_maps


def kernel(hidden_states, c_attn_w, c_attn_b, c_proj_w, c_proj_b):
    from concourse import bass_utils

    nc = _build_nc()
    in_maps = _make_in_maps(hidden_states, c_attn_w, c_attn_b, c_proj_w)
    res = bass_utils.run_bass_kernel_spmd(nc, in_maps, core_ids=list(range(NCORES)))
    outs = [np.asarray(r["out"], dtype=np.float32) for r in res.results]
    wpb = np.asarray(c_proj_b, dtype=np.float32)
    full = np.stack(
        [sum(outs[b * GROUPS:(b + 1) * GROUPS]) + wpb for b in range(B)], axis=0)
    return full.astype(np.float32)

